# revision 8
# baseline (speedup 1.0000x reference)
"""CubicPchipKANLayer Trainium2 kernel.

Math: out[b,o] = sum_i PCHIP_interp(x[b,i]; knots y[i,:,o]) + bias[o]

Reformulation: with t = clip((x - D_MIN)/H, 0, K-1), the PCHIP interpolant is
linear over the knot tables:
    out[b,o] = sum_{i,k} phi(t[b,i]-k) * y[i,o,k] + psi(t[b,i]-k) * H*m[i,o,k]
with phi(s) = r^2(3-2r), psi(s) = s*r^2, r = relu(1-|s|);  m = pchip slopes
(depend only on the parameter y, precomputed host-side).  The device computes
a dense (B x 2*D_IN*K) @ (2*D_IN*K x D_OUT) matmul whose left factor is built
on-chip from x.

Sharding: contraction-parallel over d_in — core c owns i in [32c, 32c+32).
Host sums the 8 partial (512, 256) outputs and adds bias.

Device pipeline per i-pair j (16 pairs):
  PE  : broadcast t rows (2,512)->(128,512) via ones-matmul (exact fp32)
  DVE : s = t_bc - k  (tensor_scalar, per-partition k column)
  ACT : |s| ; r = relu(1-|s|) ; r2 = r^2
  DVE : phi'' = (r-1.5)*r2  [pairs with -2y table] ; psi = s*r2 [pairs w/ H*m]
  PE  : 4 b-tiles x 2 matmuls accumulate into PSUM (128,256) x4

NOTE: this walrus build allows only ONE semaphore wait per instruction; the
kernel stages all DMA loads up front and places an all-engine barrier after
them so no downstream instruction ever needs a DMA wait, and the compute loop
is ordered so each instruction needs at most one cross-engine wait.
"""
import sys
sys.path.insert(0, '/opt/trn_rl_repo')
import numpy as np

B, D_IN, D_OUT, K = 512, 256, 256, 64
D_MIN, D_MAX = -2.0, 2.0
H = (D_MAX - D_MIN) / (K - 1)
N_CORES = 8
I_PER = D_IN // N_CORES          # 32 d_in rows per core
NPAIR = I_PER // 2               # 16 i-pairs per core

# main-matmul dtype: "f32" (exact, 4 cyc/row) or "f32r" (1 cyc/row, reduced mantissa)
MAIN_DT = "f32r"
F32R_TRUNC_BITS = 10  # low mantissa bits zeroed on the f32r grid (empirical)

_CACHE = {}


def _pchip_hm(y):
    """H * pchip_slopes(y), float64 internally, mirroring reference._pchip_slopes."""
    y = y.astype(np.float64)
    delta = (y[..., 1:] - y[..., :-1]) / H
    d0, d1 = delta[..., :-1], delta[..., 1:]
    denom = d0 + d1
    small = np.abs(denom) < 1e-12
    hm = 2.0 * d0 * d1 / np.where(small, 1.0, denom)
    hm = np.where(small, 0.0, hm)
    m_inner = np.where(d0 * d1 > 0, hm, 0.0)
    m0 = (3.0 * delta[..., 0] - delta[..., 1]) / 2.0
    mN = (3.0 * delta[..., -1] - delta[..., -2]) / 2.0
    m0 = np.where(m0 * delta[..., 0] <= 0, 0.0, m0)
    mN = np.where(mN * delta[..., -1] <= 0, 0.0, mN)
    cond0 = (delta[..., 0] * delta[..., 1] < 0) & (np.abs(m0) > np.abs(3.0 * delta[..., 0]))
    m0 = np.where(cond0, 3.0 * delta[..., 0], m0)
    condN = (delta[..., -1] * delta[..., -2] < 0) & (np.abs(mN) > np.abs(3.0 * delta[..., -1]))
    mN = np.where(condN, 3.0 * delta[..., -1], mN)
    m = np.concatenate([m0[..., None], m_inner, mN[..., None]], axis=-1)
    return (H * m).astype(np.float32)


def _round_f32r(a):
    """Round fp32 onto the f32r grid (truncate low mantissa bits)."""
    if F32R_TRUNC_BITS == 0:
        return a
    mask = np.uint32(0xFFFFFFFF) << np.uint32(F32R_TRUNC_BITS)
    return (a.view(np.uint32) & mask).view(np.float32)


def _build_tables(y):
    """Per-core stacked rhs tables, shape (N_CORES, 2*NPAIR, 128, D_OUT).

    Chunk 2j   = -2*y rows  (64 k-rows of i0 | 64 k-rows of i1)
    Chunk 2j+1 = H*m rows   (same row layout)
    """
    hm = _pchip_hm(y)                                       # (d_in, d_out, K)
    y2 = (-2.0 * y.astype(np.float64)).astype(np.float32)
    y2_t = np.ascontiguousarray(np.transpose(y2, (0, 2, 1)))  # (d_in, K, d_out)
    hm_t = np.ascontiguousarray(np.transpose(hm, (0, 2, 1)))
    tbl = np.empty((N_CORES, NPAIR, 2, 2, K, D_OUT), np.float32)
    for c in range(N_CORES):
        i0 = c * I_PER
        tbl[c, :, 0] = y2_t[i0:i0 + I_PER].reshape(NPAIR, 2, K, D_OUT)
        tbl[c, :, 1] = hm_t[i0:i0 + I_PER].reshape(NPAIR, 2, K, D_OUT)
    tbl = tbl.reshape(N_CORES, 2 * NPAIR, 2 * K, D_OUT)
    if MAIN_DT == "f32r":
        tbl = _round_f32r(tbl)
    return np.ascontiguousarray(tbl)


def _build_bass():
    import concourse.bass as bass
    import concourse.tile as tile
    from concourse import mybir
    from concourse.tile import add_dep_helper

    F32 = mybir.dt.float32
    DT = mybir.dt.float32r if MAIN_DT == "f32r" else F32
    ACTF = mybir.ActivationFunctionType
    ALU = mybir.AluOpType

    nc = bass.Bass()
    xt_d = nc.dram_tensor("xt", [2, NPAIR * B], F32, kind="ExternalInput")
    tbl_d = nc.dram_tensor("tbl", [2 * NPAIR, 2 * K, D_OUT], DT, kind="ExternalInput")
    e_d = nc.dram_tensor("ones2", [2, 128], F32, kind="ExternalInput")
    nk_d = nc.dram_tensor("nkcol", [128, 1], F32, kind="ExternalInput")
    tc_d = nc.dram_tensor("tcol", [2, 1], F32, kind="ExternalInput")
    out_d = nc.dram_tensor("out", [B, D_OUT], F32, kind="ExternalOutput")

    with tile.TileContext(nc) as tc:
        with tc.tile_pool(name="const", bufs=1) as cpool, \
             tc.tile_pool(name="tb", bufs=1) as tbpool, \
             tc.tile_pool(name="wk", bufs=3) as wkpool, \
             tc.tile_pool(name="wt", bufs=3) as wtpool, \
             tc.tile_pool(name="res", bufs=1) as respool, \
             tc.tile_pool(name="pacc", bufs=1, space="PSUM") as paccpool, \
             tc.tile_pool(name="pbc", bufs=2, space="PSUM") as pbcpool:

            # ---- stage all loads, then barrier (erases DMA waits downstream)
            e_t = cpool.tile([2, 128], F32)
            nk_t = cpool.tile([128, 1], F32)
            tc_t = cpool.tile([2, 1], F32)
            xt_t = cpool.tile([2, NPAIR * B], F32)
            nc.sync.dma_start(e_t[:], e_d[:])
            nc.sync.dma_start(nk_t[:], nk_d[:])
            nc.sync.dma_start(tc_t[:], tc_d[:])
            nc.sync.dma_start(xt_t[:], xt_d[:])
            tb_t = []
            dma_insts = []
            for j in range(2 * NPAIR):
                ct = tbpool.tile([2 * K, D_OUT], DT, tag=f"tb{j}", name=f"tb{j}")
                dma_insts.append(nc.sync.dma_start(ct[:], tbl_d[j]))
                tb_t.append(ct)
            # absorb the 8 round-robined HWDGE semaphore lanes one nop at a
            # time (this walrus build allows a single wait per instruction),
            # then a strict barrier that every later instruction hangs off.
            for idx in range(8):
                n = nc.sync.nop()
                add_dep_helper(n.ins, dma_insts[-(idx + 1)].ins, sync=True,
                               reason="dma lane absorber")
            tc.strict_bb_all_engine_barrier()

            # ---- t = clip(x/H - D_MIN/H, 0, K-1)
            t_t = cpool.tile([2, NPAIR * B], F32)
            nc.scalar.activation(t_t[:], xt_t[:], ACTF.Relu,
                                 bias=tc_t[:], scale=1.0 / H)
            nc.vector.tensor_scalar_min(t_t[:], t_t[:], float(K - 1))

            acc = [paccpool.tile([128, D_OUT], F32, tag=f"acc{bt}", name=f"acc{bt}")
                   for bt in range(4)]

            for j in range(NPAIR):
                # broadcast t rows (2j, 2j+1) across partition halves
                bacc = pbcpool.tile([128, B], F32, tag="bc")
                nc.tensor.matmul(bacc[:], e_t[:], t_t[:, j * B:(j + 1) * B],
                                 start=True, stop=True)

                # s = t_bc - (k mod 64)   (DVE, reads PSUM + per-partition col)
                s_t = wkpool.tile([128, B], F32, tag="s")
                nc.vector.tensor_scalar(s_t[:], bacc[:], nk_t[:], None, op0=ALU.add)

                ab_t = wkpool.tile([128, B], F32, tag="ab")
                nc.scalar.activation(ab_t[:], s_t[:], ACTF.Abs)
                r_t = wkpool.tile([128, B], F32, tag="r")
                nc.scalar.activation(r_t[:], ab_t[:], ACTF.Relu, bias=1.0, scale=-1.0)
                r2_t = wkpool.tile([128, B], F32, tag="r2")
                nc.scalar.activation(r2_t[:], r_t[:], ACTF.Square)

                phi_t = wtpool.tile([128, B], DT, tag="phi")
                nc.vector.scalar_tensor_tensor(phi_t[:], r_t[:], -1.5, r2_t[:],
                                               op0=ALU.add, op1=ALU.mult)
                psi_t = wtpool.tile([128, B], DT, tag="psi")
                nc.vector.tensor_mul(psi_t[:], s_t[:], r2_t[:])

                ty_t, tm_t = tb_t[2 * j], tb_t[2 * j + 1]
                for bt in range(4):
                    bs = slice(bt * 128, (bt + 1) * 128)
                    # psi first: its DVE tick covers phi's for the wait
                    nc.tensor.matmul(acc[bt][:], psi_t[:, bs], tm_t[:],
                                     start=(j == 0), stop=False)
                    nc.tensor.matmul(acc[bt][:], phi_t[:, bs], ty_t[:],
                                     start=False, stop=(j == NPAIR - 1))

            for bt in range(4):
                o_t = respool.tile([128, D_OUT], F32, tag=f"o{bt}")
                nc.scalar.copy(o_t[:], acc[bt][:])
                nc.sync.dma_start(out_d[bt * 128:(bt + 1) * 128, :], o_t[:])

    return nc


def _split_multiwaits(nc):
    """Walrus in this build allows one semaphore wait per instruction.  Tile
    sometimes emits several; split the extras onto same-engine NoOps inserted
    immediately before the instruction (queue order preserves semantics)."""
    from concourse import mybir

    fix_id = 0
    for f in nc.m.functions:
        for blk in f.blocks:
            insts = blk.instructions
            out, changed = [], False
            for ins in insts:
                si = getattr(ins, "sync_info", None)
                waits = list(si.on_wait) if si and si.on_wait else []
                if len(waits) > 1:
                    for w in waits[:-1]:
                        nop = mybir.InstNoOp(name=f"I-fixw{fix_id}",
                                             engine=ins.engine)
                        fix_id += 1
                        nop.sync_info = mybir.SyncInfo(on_wait=[w], on_update=[])
                        out.append(nop)
                    ins.sync_info = mybir.SyncInfo(
                        on_wait=[waits[-1]], on_update=list(si.on_update))
                    changed = True
                out.append(ins)
            if changed:
                blk.instructions = out


def _get_compiled():
    if "nc" not in _CACHE:
        nc = _build_bass()
        _split_multiwaits(nc)
        _CACHE["nc"] = nc
    return _CACHE["nc"]


def _run(x, y, bias, trace=False):
    from concourse.bass_utils import run_bass_kernel_spmd

    x = np.asarray(x, np.float32)
    y = np.asarray(y, np.float32)
    bias = np.asarray(bias, np.float32)

    nc = _get_compiled()

    xs = np.ascontiguousarray(x.T)                     # (d_in, B)
    tbl = _build_tables(y)                             # (8, 32, 128, 256)
    e_np = np.zeros((2, 128), np.float32)
    e_np[0, :64] = 1.0
    e_np[1, 64:] = 1.0
    nk_np = (-(np.arange(128, dtype=np.float32) % 64)).reshape(128, 1)

    in_maps = []
    for c in range(N_CORES):
        in_maps.append({
            "xt": np.ascontiguousarray(
                xs[c * I_PER:(c + 1) * I_PER].reshape(NPAIR, 2, B)
                .transpose(1, 0, 2).reshape(2, NPAIR * B)),
            "tbl": tbl[c],
            "ones2": e_np,
            "nkcol": nk_np,
            "tcol": np.full((2, 1), -D_MIN / H, np.float32),
        })
    res = run_bass_kernel_spmd(nc, in_maps, core_ids=list(range(N_CORES)),
                               trace=trace)
    partial = np.stack([res.results[c]["out"] for c in range(N_CORES)])
    out = partial.astype(np.float64).sum(axis=0) + bias.astype(np.float64)
    return out.astype(np.float32), res


def kernel(x, y, bias):
    out, _ = _run(x, y, bias)
    return out


# revision 9
# speedup vs baseline: 1.2701x; 1.2701x over previous
"""CubicPchipKANLayer Trainium2 kernel.

Math: out[b,o] = sum_i PCHIP_interp(x[b,i]; knots y[i,:,o]) + bias[o]

Reformulation: with t = clip((x - D_MIN)/H, 0, K-1), the PCHIP interpolant is
linear over the knot tables:
    out[b,o] = sum_{i,k} phi(t[b,i]-k) * y[i,o,k] + psi(t[b,i]-k) * H*m[i,o,k]
with phi(s) = r^2(3-2r), psi(s) = s*r^2, r = relu(1-|s|);  m = pchip slopes
(functions of the parameter y only, precomputed host-side).  The device
computes a dense (2*D_IN*K x B) weight matrix on-chip from x and contracts it
with the (2*D_IN*K x D_OUT) tables on the PE.

Sharding: contraction-parallel over d_in — core c owns i in [32c, 32c+32).
Host sums the 8 partial (D_OUT, B) outputs, transposes, adds bias.

Device pipeline per i-pair j (16 pairs of d_in rows):
  PE  : s = E_j^T @ [t; ones]  — one (c=33) matmul broadcasts the pair's two
        t rows across partition halves AND subtracts k (E carries a -k row).
  ACT : |s| ; r = relu(1-|s|) ; r2 = r^2
  DVE : phi'' = (r-1.5)*r2  [pairs with -2y table] ; psi = s*r2 [pairs w/ H*m]
  PE  : 4 accumulating matmuls, tables stationary (128x128), W moving (N=512),
        into two (o_half, B) PSUM accumulators (output kept transposed).

NOTE: this walrus build allows only ONE semaphore wait per instruction; a
post-scheduling pass splits extra waits onto same-engine NoOps.
"""
import sys
sys.path.insert(0, '/opt/trn_rl_repo')
import numpy as np

B, D_IN, D_OUT, K = 512, 256, 256, 64
D_MIN, D_MAX = -2.0, 2.0
H = (D_MAX - D_MIN) / (K - 1)
N_CORES = 8
I_PER = D_IN // N_CORES          # 32 d_in rows per core
NPAIR = I_PER // 2               # 16 i-pairs per core

# main-matmul dtype: "f32" (exact, 4 cyc/row) or "f32r" (1 cyc/row, reduced mantissa)
MAIN_DT = "f32r"
F32R_TRUNC_BITS = 10  # low mantissa bits zeroed on the f32r grid (empirical)

_CACHE = {}


def _pchip_hm(y):
    """H * pchip_slopes(y), float64 internally, mirroring reference._pchip_slopes."""
    y = y.astype(np.float64)
    delta = (y[..., 1:] - y[..., :-1]) / H
    d0, d1 = delta[..., :-1], delta[..., 1:]
    denom = d0 + d1
    small = np.abs(denom) < 1e-12
    hm = 2.0 * d0 * d1 / np.where(small, 1.0, denom)
    hm = np.where(small, 0.0, hm)
    m_inner = np.where(d0 * d1 > 0, hm, 0.0)
    m0 = (3.0 * delta[..., 0] - delta[..., 1]) / 2.0
    mN = (3.0 * delta[..., -1] - delta[..., -2]) / 2.0
    m0 = np.where(m0 * delta[..., 0] <= 0, 0.0, m0)
    mN = np.where(mN * delta[..., -1] <= 0, 0.0, mN)
    cond0 = (delta[..., 0] * delta[..., 1] < 0) & (np.abs(m0) > np.abs(3.0 * delta[..., 0]))
    m0 = np.where(cond0, 3.0 * delta[..., 0], m0)
    condN = (delta[..., -1] * delta[..., -2] < 0) & (np.abs(mN) > np.abs(3.0 * delta[..., -1]))
    mN = np.where(condN, 3.0 * delta[..., -1], mN)
    m = np.concatenate([m0[..., None], m_inner, mN[..., None]], axis=-1)
    return (H * m).astype(np.float32)


def _round_f32r(a):
    """Round fp32 onto the f32r grid (truncate low mantissa bits)."""
    if F32R_TRUNC_BITS == 0:
        return a
    mask = np.uint32(0xFFFFFFFF) << np.uint32(F32R_TRUNC_BITS)
    return (a.view(np.uint32) & mask).view(np.float32)


def _build_tables(y):
    """Per-core rhs tables, shape (N_CORES, 2*K, 2*NPAIR, D_OUT).

    Table column group (j, h): h=0 -> -2*y rows for pair j, h=1 -> H*m rows.
    Row layout within a group: 64 k-rows of i0 then 64 k-rows of i1.
    Device loads this as a (128, 2*NPAIR*D_OUT) tile (32KB/partition,
    fully contiguous rows for DMA efficiency).
    """
    hm = _pchip_hm(y)                                       # (d_in, d_out, K)
    y2 = (-2.0 * y.astype(np.float64)).astype(np.float32)
    y2_t = np.ascontiguousarray(np.transpose(y2, (0, 2, 1)))  # (d_in, K, d_out)
    hm_t = np.ascontiguousarray(np.transpose(hm, (0, 2, 1)))
    tbl = np.empty((N_CORES, NPAIR, 2, 2, K, D_OUT), np.float32)
    for c in range(N_CORES):
        i0 = c * I_PER
        tbl[c, :, 0] = y2_t[i0:i0 + I_PER].reshape(NPAIR, 2, K, D_OUT)
        tbl[c, :, 1] = hm_t[i0:i0 + I_PER].reshape(NPAIR, 2, K, D_OUT)
    # (c, j, h, half, k, o) -> rows (half,k) x cols (j,h,o)
    tbl = tbl.transpose(0, 3, 4, 1, 2, 5).reshape(N_CORES, 2 * K, 2 * NPAIR * D_OUT)
    if MAIN_DT == "f32r":
        tbl = _round_f32r(tbl)
    return np.ascontiguousarray(tbl)


def _build_selector():
    """E (33, NPAIR*128) fp32: per pair j a (33,128) stationary block.
    Row c<32: 1.0 where (p<64 and c==2j) or (p>=64 and c==2j+1).
    Row 32:   -(p mod 64)  (the -k term; pairs with the ones-row of t)."""
    e = np.zeros((33, NPAIR * 128), np.float32)
    for j in range(NPAIR):
        e[2 * j, j * 128:j * 128 + 64] = 1.0
        e[2 * j + 1, j * 128 + 64:(j + 1) * 128] = 1.0
    e[32] = np.tile(-(np.arange(128, dtype=np.float32) % 64), NPAIR)
    return e


def _build_bass():
    import concourse.bass as bass
    import concourse.tile as tile
    from concourse import mybir

    F32 = mybir.dt.float32
    DT = mybir.dt.float32r if MAIN_DT == "f32r" else F32
    ACTF = mybir.ActivationFunctionType
    ALU = mybir.AluOpType
    TW = 2 * NPAIR * D_OUT            # 8192 table columns

    nc = bass.Bass()
    xt_d = nc.dram_tensor("xt", [33, B], F32, kind="ExternalInput")
    tbl_d = nc.dram_tensor("tbl", [2 * K, TW], DT, kind="ExternalInput")
    e_d = nc.dram_tensor("sel", [33, NPAIR * 128], F32, kind="ExternalInput")
    tc_d = nc.dram_tensor("tcol", [33, 1], F32, kind="ExternalInput")
    out_d = nc.dram_tensor("out", [D_OUT, B], F32, kind="ExternalOutput")

    with tile.TileContext(nc) as tc:
        with tc.tile_pool(name="const", bufs=1) as cpool, \
             tc.tile_pool(name="wk", bufs=3) as wkpool, \
             tc.tile_pool(name="wt", bufs=3) as wtpool, \
             tc.tile_pool(name="res", bufs=1) as respool, \
             tc.tile_pool(name="pacc", bufs=1, space="PSUM") as paccpool, \
             tc.tile_pool(name="pbc", bufs=3, space="PSUM") as pbcpool:

            e_t = cpool.tile([33, NPAIR * 128], F32)
            tc_t = cpool.tile([33, 1], F32)
            xt_t = cpool.tile([33, B], F32)
            nc.sync.dma_start(e_t[:], e_d[:])
            nc.sync.dma_start(tc_t[:], tc_d[:])
            nc.sync.dma_start(xt_t[:], xt_d[:])
            tbl_t = cpool.tile([2 * K, TW], DT)
            for p in range(8):
                w = TW // 8
                nc.sync.dma_start(tbl_t[:, p * w:(p + 1) * w],
                                  tbl_d[:, p * w:(p + 1) * w])

            # t = clip(x/H - D_MIN/H, 0, K-1); row 32 becomes exactly 1.0
            t_t = cpool.tile([33, B], F32)
            nc.scalar.activation(t_t[:], xt_t[:], ACTF.Relu,
                                 bias=tc_t[:], scale=1.0 / H)
            nc.vector.tensor_scalar_min(t_t[:], t_t[:], float(K - 1))

            accT = [paccpool.tile([128, B], F32, tag=f"accT{q}", name=f"accT{q}")
                    for q in range(2)]

            for j in range(NPAIR):
                # s = E_j^T @ [t; 1]: broadcast pair's t rows minus k, (128, B)
                bacc = pbcpool.tile([128, B], F32, tag="bc")
                nc.tensor.matmul(bacc[:], e_t[:, j * 128:(j + 1) * 128], t_t[:],
                                 start=True, stop=True)

                ab_t = wkpool.tile([128, B], F32, tag="ab")
                nc.scalar.activation(ab_t[:], bacc[:], ACTF.Abs)
                r_t = wkpool.tile([128, B], F32, tag="r")
                nc.scalar.activation(r_t[:], ab_t[:], ACTF.Relu, bias=1.0, scale=-1.0)
                r2_t = wkpool.tile([128, B], F32, tag="r2")
                nc.scalar.activation(r2_t[:], r_t[:], ACTF.Square)

                phi_t = wtpool.tile([128, B], DT, tag="phi")
                nc.vector.scalar_tensor_tensor(phi_t[:], r_t[:], -1.5, r2_t[:],
                                               op0=ALU.add, op1=ALU.mult)
                psi_t = wtpool.tile([128, B], DT, tag="psi")
                nc.vector.tensor_mul(psi_t[:], bacc[:], r2_t[:])

                for h, w_t in ((1, psi_t), (0, phi_t)):
                    base = (j * 2 + h) * D_OUT
                    for q in range(2):
                        nc.tensor.matmul(
                            accT[q][:],
                            tbl_t[:, base + q * 128: base + (q + 1) * 128],
                            w_t[:],
                            start=(j == 0 and h == 1),
                            stop=(j == NPAIR - 1 and h == 0))

            for q in range(2):
                o_t = respool.tile([128, B], F32, tag=f"o{q}", name=f"o{q}")
                nc.scalar.copy(o_t[:], accT[q][:])
                nc.sync.dma_start(out_d[q * 128:(q + 1) * 128, :], o_t[:])

    return nc


def _split_multiwaits(nc):
    """Walrus in this build allows one semaphore wait per instruction.  Tile
    sometimes emits several; split the extras onto same-engine NoOps inserted
    immediately before the instruction (queue order preserves semantics)."""
    from concourse import mybir

    fix_id = 0
    for f in nc.m.functions:
        for blk in f.blocks:
            insts = blk.instructions
            out, changed = [], False
            for ins in insts:
                si = getattr(ins, "sync_info", None)
                waits = list(si.on_wait) if si and si.on_wait else []
                if len(waits) > 1:
                    for w in waits[:-1]:
                        nop = mybir.InstNoOp(name=f"I-fixw{fix_id}",
                                             engine=ins.engine)
                        fix_id += 1
                        nop.sync_info = mybir.SyncInfo(on_wait=[w], on_update=[])
                        out.append(nop)
                    ins.sync_info = mybir.SyncInfo(
                        on_wait=[waits[-1]], on_update=list(si.on_update))
                    changed = True
                out.append(ins)
            if changed:
                blk.instructions = out


def _get_compiled():
    if "nc" not in _CACHE:
        nc = _build_bass()
        _split_multiwaits(nc)
        _CACHE["nc"] = nc
    return _CACHE["nc"]


def _run(x, y, bias, trace=False):
    from concourse.bass_utils import run_bass_kernel_spmd

    x = np.asarray(x, np.float32)
    y = np.asarray(y, np.float32)
    bias = np.asarray(bias, np.float32)

    nc = _get_compiled()

    xs = np.ascontiguousarray(x.T)                     # (d_in, B)
    tbl = _build_tables(y)                             # (8, 128, 8192)
    e_np = _build_selector()
    tc_np = np.full((33, 1), -D_MIN / H, np.float32)
    tc_np[32, 0] = 0.0

    in_maps = []
    for c in range(N_CORES):
        xt = np.empty((33, B), np.float32)
        xt[:32] = xs[c * I_PER:(c + 1) * I_PER]
        xt[32] = H                                     # relu(H/H + 0) == 1.0
        in_maps.append({
            "xt": xt,
            "tbl": tbl[c],
            "sel": e_np,
            "tcol": tc_np,
        })
    res = run_bass_kernel_spmd(nc, in_maps, core_ids=list(range(N_CORES)),
                               trace=trace)
    partialT = np.stack([res.results[c]["out"] for c in range(N_CORES)])
    out = partialT.astype(np.float64).sum(axis=0).T + bias.astype(np.float64)
    return out.astype(np.float32), res


def kernel(x, y, bias):
    out, _ = _run(x, y, bias)
    return out


# revision 10
# speedup vs baseline: 1.2704x; 1.0002x over previous
"""CubicPchipKANLayer Trainium2 kernel.

Math: out[b,o] = sum_i PCHIP_interp(x[b,i]; knots y[i,:,o]) + bias[o]

Reformulation: with t = clip((x - D_MIN)/H, 0, K-1), the PCHIP interpolant is
linear over the knot tables:
    out[b,o] = sum_{i,k} phi(t[b,i]-k) * y[i,o,k] + psi(t[b,i]-k) * H*m[i,o,k]
with phi(s) = r^2(3-2r), psi(s) = s*r^2, r = relu(1-|s|);  m = pchip slopes
(functions of the parameter y only, precomputed host-side).  The device
computes a dense (2*D_IN*K x B) weight matrix on-chip from x and contracts it
with the (2*D_IN*K x D_OUT) tables on the PE.

Sharding: contraction-parallel over d_in — core c owns i in [32c, 32c+32).
Host sums the 8 partial (D_OUT, B) outputs, transposes, adds bias.

Device pipeline per i-pair j (16 pairs of d_in rows):
  PE  : s = E_j^T @ [t; ones]  — one (c=33) matmul broadcasts the pair's two
        t rows across partition halves AND subtracts k (E carries a -k row).
  ACT : |s| ; r = relu(1-|s|) ; r2 = r^2
  DVE : phi'' = (r-1.5)*r2  [pairs with -2y table] ; psi = s*r2 [pairs w/ H*m]
  PE  : 4 accumulating matmuls, tables stationary (128x128), W moving (N=512),
        into two (o_half, B) PSUM accumulators (output kept transposed).

NOTE: this walrus build allows only ONE semaphore wait per instruction; a
post-scheduling pass splits extra waits onto same-engine NoOps.
"""
import sys
sys.path.insert(0, '/opt/trn_rl_repo')
import numpy as np

B, D_IN, D_OUT, K = 512, 256, 256, 64
D_MIN, D_MAX = -2.0, 2.0
H = (D_MAX - D_MIN) / (K - 1)
N_CORES = 8
I_PER = D_IN // N_CORES          # 32 d_in rows per core
NPAIR = I_PER // 2               # 16 i-pairs per core

# main-matmul dtype: "f32" (exact, 4 cyc/row), "f32r" (reduced mantissa),
# or "f16" (1 cyc/row, fast weight load, 10-bit mantissa)
MAIN_DT = "f16"
F32R_TRUNC_BITS = 10  # low mantissa bits zeroed on the f32r grid (empirical)

_CACHE = {}


def _pchip_hm(y):
    """H * pchip_slopes(y), float64 internally, mirroring reference._pchip_slopes."""
    y = y.astype(np.float64)
    delta = (y[..., 1:] - y[..., :-1]) / H
    d0, d1 = delta[..., :-1], delta[..., 1:]
    denom = d0 + d1
    small = np.abs(denom) < 1e-12
    hm = 2.0 * d0 * d1 / np.where(small, 1.0, denom)
    hm = np.where(small, 0.0, hm)
    m_inner = np.where(d0 * d1 > 0, hm, 0.0)
    m0 = (3.0 * delta[..., 0] - delta[..., 1]) / 2.0
    mN = (3.0 * delta[..., -1] - delta[..., -2]) / 2.0
    m0 = np.where(m0 * delta[..., 0] <= 0, 0.0, m0)
    mN = np.where(mN * delta[..., -1] <= 0, 0.0, mN)
    cond0 = (delta[..., 0] * delta[..., 1] < 0) & (np.abs(m0) > np.abs(3.0 * delta[..., 0]))
    m0 = np.where(cond0, 3.0 * delta[..., 0], m0)
    condN = (delta[..., -1] * delta[..., -2] < 0) & (np.abs(mN) > np.abs(3.0 * delta[..., -1]))
    mN = np.where(condN, 3.0 * delta[..., -1], mN)
    m = np.concatenate([m0[..., None], m_inner, mN[..., None]], axis=-1)
    return (H * m).astype(np.float32)


def _round_f32r(a):
    """Round fp32 onto the f32r grid (truncate low mantissa bits)."""
    if F32R_TRUNC_BITS == 0:
        return a
    mask = np.uint32(0xFFFFFFFF) << np.uint32(F32R_TRUNC_BITS)
    return (a.view(np.uint32) & mask).view(np.float32)


def _build_tables(y):
    """Per-core rhs tables, shape (N_CORES, 2*K, 2*NPAIR, D_OUT).

    Table column group (j, h): h=0 -> -2*y rows for pair j, h=1 -> H*m rows.
    Row layout within a group: 64 k-rows of i0 then 64 k-rows of i1.
    Device loads this as a (128, 2*NPAIR*D_OUT) tile (32KB/partition,
    fully contiguous rows for DMA efficiency).
    """
    hm = _pchip_hm(y)                                       # (d_in, d_out, K)
    y2 = (-2.0 * y.astype(np.float64)).astype(np.float32)
    y2_t = np.ascontiguousarray(np.transpose(y2, (0, 2, 1)))  # (d_in, K, d_out)
    hm_t = np.ascontiguousarray(np.transpose(hm, (0, 2, 1)))
    tbl = np.empty((N_CORES, NPAIR, 2, 2, K, D_OUT), np.float32)
    for c in range(N_CORES):
        i0 = c * I_PER
        tbl[c, :, 0] = y2_t[i0:i0 + I_PER].reshape(NPAIR, 2, K, D_OUT)
        tbl[c, :, 1] = hm_t[i0:i0 + I_PER].reshape(NPAIR, 2, K, D_OUT)
    # (c, j, h, half, k, o) -> rows (half,k) x cols (j,h,o)
    tbl = tbl.transpose(0, 3, 4, 1, 2, 5).reshape(N_CORES, 2 * K, 2 * NPAIR * D_OUT)
    if MAIN_DT == "f32r":
        tbl = _round_f32r(tbl)
    elif MAIN_DT == "f16":
        tbl = tbl.astype(np.float16)
    return np.ascontiguousarray(tbl)


def _build_selector():
    """E (33, NPAIR*128) fp32: per pair j a (33,128) stationary block.
    Row c<32: 1.0 where (p<64 and c==2j) or (p>=64 and c==2j+1).
    Row 32:   -(p mod 64)  (the -k term; pairs with the ones-row of t)."""
    e = np.zeros((33, NPAIR * 128), np.float32)
    for j in range(NPAIR):
        e[2 * j, j * 128:j * 128 + 64] = 1.0
        e[2 * j + 1, j * 128 + 64:(j + 1) * 128] = 1.0
    e[32] = np.tile(-(np.arange(128, dtype=np.float32) % 64), NPAIR)
    return e


def _build_bass():
    import concourse.bass as bass
    import concourse.tile as tile
    from concourse import mybir

    F32 = mybir.dt.float32
    DT = {"f32": F32, "f32r": mybir.dt.float32r,
          "f16": mybir.dt.float16}[MAIN_DT]
    ACTF = mybir.ActivationFunctionType
    ALU = mybir.AluOpType
    TW = 2 * NPAIR * D_OUT            # 8192 table columns

    nc = bass.Bass()
    xt_d = nc.dram_tensor("xt", [33, B], F32, kind="ExternalInput")
    tbl_d = nc.dram_tensor("tbl", [2 * K, TW], DT, kind="ExternalInput")
    e_d = nc.dram_tensor("sel", [33, NPAIR * 128], F32, kind="ExternalInput")
    tc_d = nc.dram_tensor("tcol", [33, 1], F32, kind="ExternalInput")
    out_d = nc.dram_tensor("out", [D_OUT, B], F32, kind="ExternalOutput")

    with tile.TileContext(nc) as tc:
        with tc.tile_pool(name="const", bufs=1) as cpool, \
             tc.tile_pool(name="wk", bufs=3) as wkpool, \
             tc.tile_pool(name="wt", bufs=3) as wtpool, \
             tc.tile_pool(name="res", bufs=1) as respool, \
             tc.tile_pool(name="pacc", bufs=1, space="PSUM") as paccpool, \
             tc.tile_pool(name="pbc", bufs=3, space="PSUM") as pbcpool:

            e_t = cpool.tile([33, NPAIR * 128], F32)
            tc_t = cpool.tile([33, 1], F32)
            xt_t = cpool.tile([33, B], F32)
            # smalls go through SWDGE so the big table loads can't queue
            # ahead of them on the HWDGE rings
            nc.gpsimd.dma_start(e_t[:], e_d[:])
            nc.gpsimd.dma_start(tc_t[:], tc_d[:])
            nc.gpsimd.dma_start(xt_t[:], xt_d[:])
            tbl_t = cpool.tile([2 * K, TW], DT)
            for p in range(8):
                w = TW // 8
                nc.sync.dma_start(tbl_t[:, p * w:(p + 1) * w],
                                  tbl_d[:, p * w:(p + 1) * w])

            # t = clip(x/H - D_MIN/H, 0, K-1); row 32 becomes exactly 1.0
            t_t = cpool.tile([33, B], F32)
            nc.scalar.activation(t_t[:], xt_t[:], ACTF.Relu,
                                 bias=tc_t[:], scale=1.0 / H)
            nc.vector.tensor_scalar_min(t_t[:], t_t[:], float(K - 1))

            accT = [paccpool.tile([128, B], F32, tag=f"accT{q}", name=f"accT{q}")
                    for q in range(2)]

            for j in range(NPAIR):
                # s = E_j^T @ [t; 1]: broadcast pair's t rows minus k, (128, B)
                bacc = pbcpool.tile([128, B], F32, tag="bc")
                nc.tensor.matmul(bacc[:], e_t[:, j * 128:(j + 1) * 128], t_t[:],
                                 start=True, stop=True)

                ab_t = wkpool.tile([128, B], F32, tag="ab")
                nc.scalar.activation(ab_t[:], bacc[:], ACTF.Abs)
                r_t = wkpool.tile([128, B], F32, tag="r")
                nc.scalar.activation(r_t[:], ab_t[:], ACTF.Relu, bias=1.0, scale=-1.0)
                r2_t = wkpool.tile([128, B], F32, tag="r2")
                nc.scalar.activation(r2_t[:], r_t[:], ACTF.Square)

                phi_t = wtpool.tile([128, B], DT, tag="phi")
                nc.vector.scalar_tensor_tensor(phi_t[:], r_t[:], -1.5, r2_t[:],
                                               op0=ALU.add, op1=ALU.mult)
                psi_t = wtpool.tile([128, B], DT, tag="psi")
                nc.vector.tensor_mul(psi_t[:], bacc[:], r2_t[:])

                for h, w_t in ((1, psi_t), (0, phi_t)):
                    base = (j * 2 + h) * D_OUT
                    for q in range(2):
                        nc.tensor.matmul(
                            accT[q][:],
                            tbl_t[:, base + q * 128: base + (q + 1) * 128],
                            w_t[:],
                            start=(j == 0 and h == 1),
                            stop=(j == NPAIR - 1 and h == 0))

            for q in range(2):
                o_t = respool.tile([128, B], F32, tag=f"o{q}", name=f"o{q}")
                nc.scalar.copy(o_t[:], accT[q][:])
                nc.sync.dma_start(out_d[q * 128:(q + 1) * 128, :], o_t[:])

    return nc


def _split_multiwaits(nc):
    """Walrus in this build allows one semaphore wait per instruction.  Tile
    sometimes emits several; split the extras onto same-engine NoOps inserted
    immediately before the instruction (queue order preserves semantics)."""
    from concourse import mybir

    fix_id = 0
    for f in nc.m.functions:
        for blk in f.blocks:
            insts = blk.instructions
            out, changed = [], False
            for ins in insts:
                si = getattr(ins, "sync_info", None)
                waits = list(si.on_wait) if si and si.on_wait else []
                if len(waits) > 1:
                    for w in waits[:-1]:
                        nop = mybir.InstNoOp(name=f"I-fixw{fix_id}",
                                             engine=ins.engine)
                        fix_id += 1
                        nop.sync_info = mybir.SyncInfo(on_wait=[w], on_update=[])
                        out.append(nop)
                    ins.sync_info = mybir.SyncInfo(
                        on_wait=[waits[-1]], on_update=list(si.on_update))
                    changed = True
                out.append(ins)
            if changed:
                blk.instructions = out


def _get_compiled():
    if "nc" not in _CACHE:
        nc = _build_bass()
        _split_multiwaits(nc)
        _CACHE["nc"] = nc
    return _CACHE["nc"]


def _run(x, y, bias, trace=False):
    from concourse.bass_utils import run_bass_kernel_spmd

    x = np.asarray(x, np.float32)
    y = np.asarray(y, np.float32)
    bias = np.asarray(bias, np.float32)

    nc = _get_compiled()

    xs = np.ascontiguousarray(x.T)                     # (d_in, B)
    tbl = _build_tables(y)                             # (8, 128, 8192)
    e_np = _build_selector()
    tc_np = np.full((33, 1), -D_MIN / H, np.float32)
    tc_np[32, 0] = 0.0

    in_maps = []
    for c in range(N_CORES):
        xt = np.empty((33, B), np.float32)
        xt[:32] = xs[c * I_PER:(c + 1) * I_PER]
        xt[32] = H                                     # relu(H/H + 0) == 1.0
        in_maps.append({
            "xt": xt,
            "tbl": tbl[c],
            "sel": e_np,
            "tcol": tc_np,
        })
    res = run_bass_kernel_spmd(nc, in_maps, core_ids=list(range(N_CORES)),
                               trace=trace)
    partialT = np.stack([res.results[c]["out"] for c in range(N_CORES)])
    out = partialT.astype(np.float64).sum(axis=0).T + bias.astype(np.float64)
    return out.astype(np.float32), res


def kernel(x, y, bias):
    out, _ = _run(x, y, bias)
    return out


# revision 12
# speedup vs baseline: 1.3870x; 1.0918x over previous
"""CubicPchipKANLayer Trainium2 kernel.

Math: out[b,o] = sum_i PCHIP_interp(x[b,i]; knots y[i,:,o]) + bias[o]

Reformulation: with t = clip((x - D_MIN)/H, 0, K-1), the PCHIP interpolant is
linear over the knot tables:
    out[b,o] = sum_{i,k} phi(t[b,i]-k) * y[i,o,k] + psi(t[b,i]-k) * H*m[i,o,k]
with phi(s) = r^2(3-2r), psi(s) = s*r^2, r = relu(1-|s|);  m = pchip slopes
(functions of the parameter y only, precomputed host-side).  The device
computes a dense (2*D_IN*K x B) weight matrix on-chip from x and contracts it
with the (2*D_IN*K x D_OUT) tables on the PE.

Sharding: contraction-parallel over d_in — core c owns i in [32c, 32c+32).
Host sums the 8 partial (D_OUT, B) outputs, transposes, adds bias.

Device pipeline per i-pair j (16 pairs of d_in rows):
  PE  : s = E_j^T @ [t; ones]  — one (c=33) matmul broadcasts the pair's two
        t rows across partition halves AND subtracts k (E carries a -k row).
  ACT : |s| ; r = relu(1-|s|) ; r2 = r^2
  DVE : phi'' = (r-1.5)*r2  [pairs with -2y table] ; psi = s*r2 [pairs w/ H*m]
  PE  : 4 accumulating matmuls, tables stationary (128x128), W moving (N=512),
        into two (o_half, B) PSUM accumulators (output kept transposed).

NOTE: this walrus build allows only ONE semaphore wait per instruction; a
post-scheduling pass splits extra waits onto same-engine NoOps.
"""
import sys
sys.path.insert(0, '/opt/trn_rl_repo')
import numpy as np

B, D_IN, D_OUT, K = 512, 256, 256, 64
D_MIN, D_MAX = -2.0, 2.0
H = (D_MAX - D_MIN) / (K - 1)
N_CORES = 8
I_PER = D_IN // N_CORES          # 32 d_in rows per core
NPAIR = I_PER // 2               # 16 i-pairs per core

# main-matmul dtype: "f32" (exact, 4 cyc/row), "f32r" (reduced mantissa),
# or "f16" (1 cyc/row, fast weight load, 10-bit mantissa)
MAIN_DT = "f16"
F32R_TRUNC_BITS = 10  # low mantissa bits zeroed on the f32r grid (empirical)

_CACHE = {}


def _pchip_hm(y):
    """H * pchip_slopes(y), float64 internally, mirroring reference._pchip_slopes."""
    y = y.astype(np.float64)
    delta = (y[..., 1:] - y[..., :-1]) / H
    d0, d1 = delta[..., :-1], delta[..., 1:]
    denom = d0 + d1
    small = np.abs(denom) < 1e-12
    hm = 2.0 * d0 * d1 / np.where(small, 1.0, denom)
    hm = np.where(small, 0.0, hm)
    m_inner = np.where(d0 * d1 > 0, hm, 0.0)
    m0 = (3.0 * delta[..., 0] - delta[..., 1]) / 2.0
    mN = (3.0 * delta[..., -1] - delta[..., -2]) / 2.0
    m0 = np.where(m0 * delta[..., 0] <= 0, 0.0, m0)
    mN = np.where(mN * delta[..., -1] <= 0, 0.0, mN)
    cond0 = (delta[..., 0] * delta[..., 1] < 0) & (np.abs(m0) > np.abs(3.0 * delta[..., 0]))
    m0 = np.where(cond0, 3.0 * delta[..., 0], m0)
    condN = (delta[..., -1] * delta[..., -2] < 0) & (np.abs(mN) > np.abs(3.0 * delta[..., -1]))
    mN = np.where(condN, 3.0 * delta[..., -1], mN)
    m = np.concatenate([m0[..., None], m_inner, mN[..., None]], axis=-1)
    return (H * m).astype(np.float32)


def _round_f32r(a):
    """Round fp32 onto the f32r grid (truncate low mantissa bits)."""
    if F32R_TRUNC_BITS == 0:
        return a
    mask = np.uint32(0xFFFFFFFF) << np.uint32(F32R_TRUNC_BITS)
    return (a.view(np.uint32) & mask).view(np.float32)


def _build_tables(y):
    """Per-core rhs tables, shape (N_CORES, 2*K, 2*NPAIR, D_OUT).

    Table column group (j, h): h=0 -> -2*y rows for pair j, h=1 -> H*m rows.
    Row layout within a group: 64 k-rows of i0 then 64 k-rows of i1.
    Device loads this as a (128, 2*NPAIR*D_OUT) tile (32KB/partition,
    fully contiguous rows for DMA efficiency).
    """
    hm = _pchip_hm(y)                                       # (d_in, d_out, K)
    y2 = (-2.0 * y.astype(np.float64)).astype(np.float32)
    y2_t = np.ascontiguousarray(np.transpose(y2, (0, 2, 1)))  # (d_in, K, d_out)
    hm_t = np.ascontiguousarray(np.transpose(hm, (0, 2, 1)))
    tbl = np.empty((N_CORES, NPAIR, 2, 2, K, D_OUT), np.float32)
    for c in range(N_CORES):
        i0 = c * I_PER
        tbl[c, :, 0] = y2_t[i0:i0 + I_PER].reshape(NPAIR, 2, K, D_OUT)
        tbl[c, :, 1] = hm_t[i0:i0 + I_PER].reshape(NPAIR, 2, K, D_OUT)
    # (c, j, h, half, k, o) -> rows (half,k) x cols (j,h,o)
    tbl = tbl.transpose(0, 3, 4, 1, 2, 5).reshape(N_CORES, 2 * K, 2 * NPAIR * D_OUT)
    if MAIN_DT == "f32r":
        tbl = _round_f32r(tbl)
    elif MAIN_DT == "f16":
        tbl = tbl.astype(np.float16)
    return np.ascontiguousarray(tbl)


def _build_selector():
    """E (66, NPAIR*128) fp16: per pair j a (66,128) stationary block.
    Rows 0-31 select t_hi rows (1.0 where (p<64, c==2j) or (p>=64, c==2j+1)),
    rows 32-63 repeat the selector for the t_lo rows, row 64 is -(p mod 64)
    (pairs with the ones-row).  All entries are fp16-exact (ints <= 63)."""
    e = np.zeros((65, NPAIR * 128), np.float16)
    for j in range(NPAIR):
        e[2 * j, j * 128:j * 128 + 64] = 1.0
        e[2 * j + 1, j * 128 + 64:(j + 1) * 128] = 1.0
        e[32 + 2 * j, j * 128:j * 128 + 64] = 1.0
        e[32 + 2 * j + 1, j * 128 + 64:(j + 1) * 128] = 1.0
    e[64] = np.tile(-(np.arange(128, dtype=np.float16) % 64), NPAIR)
    return e


def _build_bass():
    import concourse.bass as bass
    import concourse.tile as tile
    from concourse import mybir

    F32 = mybir.dt.float32
    DT = {"f32": F32, "f32r": mybir.dt.float32r,
          "f16": mybir.dt.float16}[MAIN_DT]
    ACTF = mybir.ActivationFunctionType
    ALU = mybir.AluOpType
    TW = 2 * NPAIR * D_OUT            # 8192 table columns

    F16 = mybir.dt.float16
    nc = bass.Bass()
    xt_d = nc.dram_tensor("xt", [33, B], F32, kind="ExternalInput")
    tbl_d = nc.dram_tensor("tbl", [2 * K, TW], DT, kind="ExternalInput")
    e_d = nc.dram_tensor("sel", [65, NPAIR * 128], F16, kind="ExternalInput")
    tc_d = nc.dram_tensor("tcol", [33, 1], F32, kind="ExternalInput")
    out_d = nc.dram_tensor("out", [D_OUT, B], F32, kind="ExternalOutput")

    with tile.TileContext(nc) as tc:
        with tc.tile_pool(name="const", bufs=1) as cpool, \
             tc.tile_pool(name="wk", bufs=3) as wkpool, \
             tc.tile_pool(name="wt", bufs=3) as wtpool, \
             tc.tile_pool(name="res", bufs=1) as respool, \
             tc.tile_pool(name="pacc", bufs=1, space="PSUM") as paccpool, \
             tc.tile_pool(name="pbc", bufs=3, space="PSUM") as pbcpool:

            e_t = cpool.tile([65, NPAIR * 128], F16)
            tc_t = cpool.tile([33, 1], F32)
            xt_t = cpool.tile([33, B], F32)
            # smalls go through SWDGE so the big table loads can't queue
            # ahead of them on the HWDGE rings; xt/tc first (t-prep gate)
            nc.gpsimd.dma_start(xt_t[:], xt_d[:])
            nc.gpsimd.dma_start(tc_t[:], tc_d[:])
            nc.gpsimd.dma_start(e_t[:], e_d[:])
            tbl_t = cpool.tile([2 * K, TW], DT)
            for p in range(8):
                w = TW // 8
                nc.sync.dma_start(tbl_t[:, p * w:(p + 1) * w],
                                  tbl_d[:, p * w:(p + 1) * w])

            # pre-warm the PE so the HAM clock gate opens before real work
            warm_t = cpool.tile([128, B], F16)
            nc.gpsimd.memset(warm_t[:], 0.0)
            wacc = pbcpool.tile([128, B], F32, tag="bc", name="wacc")
            for _ in range(10):
                nc.tensor.matmul(wacc[:], warm_t[:, :128], warm_t[:],
                                 start=True, stop=True)

            # t = clip(x/H - D_MIN/H, 0, K-1); row 32 becomes exactly 1.0
            t_t = cpool.tile([33, B], F32)
            nc.scalar.activation(t_t[:], xt_t[:], ACTF.Relu,
                                 bias=tc_t[:], scale=1.0 / H)
            nc.vector.tensor_scalar_min(t_t[:], t_t[:], float(K - 1))
            # split t into fp16 hi + lo halves of a (65, B) rhs; the c=65
            # broadcast matmul then reconstructs t to ~2^-22 while running
            # at fp16 speed (single pass, fast weight load).  Rows: 0-31 hi,
            # 32-63 lo, 64 ones (for the -k term).
            t2_t = cpool.tile([65, B], F16)
            nc.scalar.copy(t2_t[0:32, :], t_t[0:32, :])
            nc.scalar.copy(t2_t[64:65, :], t_t[32:33, :])
            nc.vector.tensor_sub(t2_t[32:64, :], t_t[0:32, :], t2_t[0:32, :])

            accT = [paccpool.tile([128, B], F32, tag=f"accT{q}", name=f"accT{q}")
                    for q in range(2)]

            for j in range(NPAIR):
                # s = E_j^T @ [t; 1]: broadcast pair's t rows minus k, (128, B)
                bacc = pbcpool.tile([128, B], F32, tag="bc")
                nc.tensor.matmul(bacc[:], e_t[:, j * 128:(j + 1) * 128], t2_t[:],
                                 start=True, stop=True)

                ab_t = wkpool.tile([128, B], F32, tag="ab")
                nc.scalar.activation(ab_t[:], bacc[:], ACTF.Abs)
                r_t = wkpool.tile([128, B], F32, tag="r")
                nc.scalar.activation(r_t[:], ab_t[:], ACTF.Relu, bias=1.0, scale=-1.0)
                r2_t = wkpool.tile([128, B], F32, tag="r2")
                nc.scalar.activation(r2_t[:], r_t[:], ACTF.Square)

                phi_t = wtpool.tile([128, B], DT, tag="phi")
                nc.vector.scalar_tensor_tensor(phi_t[:], r_t[:], -1.5, r2_t[:],
                                               op0=ALU.add, op1=ALU.mult)
                psi_t = wtpool.tile([128, B], DT, tag="psi")
                nc.vector.tensor_mul(psi_t[:], bacc[:], r2_t[:])

                for h, w_t in ((1, psi_t), (0, phi_t)):
                    base = (j * 2 + h) * D_OUT
                    for q in range(2):
                        nc.tensor.matmul(
                            accT[q][:],
                            tbl_t[:, base + q * 128: base + (q + 1) * 128],
                            w_t[:],
                            start=(j == 0 and h == 1),
                            stop=(j == NPAIR - 1 and h == 0))

            for q in range(2):
                o_t = respool.tile([128, B], F32, tag=f"o{q}", name=f"o{q}")
                nc.scalar.copy(o_t[:], accT[q][:])
                nc.sync.dma_start(out_d[q * 128:(q + 1) * 128, :], o_t[:])

    return nc


def _split_multiwaits(nc):
    """Walrus in this build allows one semaphore wait per instruction.  Tile
    sometimes emits several; split the extras onto same-engine NoOps inserted
    immediately before the instruction (queue order preserves semantics)."""
    from concourse import mybir

    fix_id = 0
    for f in nc.m.functions:
        for blk in f.blocks:
            insts = blk.instructions
            out, changed = [], False
            for ins in insts:
                si = getattr(ins, "sync_info", None)
                waits = list(si.on_wait) if si and si.on_wait else []
                if len(waits) > 1:
                    for w in waits[:-1]:
                        nop = mybir.InstNoOp(name=f"I-fixw{fix_id}",
                                             engine=ins.engine)
                        fix_id += 1
                        nop.sync_info = mybir.SyncInfo(on_wait=[w], on_update=[])
                        out.append(nop)
                    ins.sync_info = mybir.SyncInfo(
                        on_wait=[waits[-1]], on_update=list(si.on_update))
                    changed = True
                out.append(ins)
            if changed:
                blk.instructions = out


def _get_compiled():
    if "nc" not in _CACHE:
        nc = _build_bass()
        _split_multiwaits(nc)
        _CACHE["nc"] = nc
    return _CACHE["nc"]


def _run(x, y, bias, trace=False):
    from concourse.bass_utils import run_bass_kernel_spmd

    x = np.asarray(x, np.float32)
    y = np.asarray(y, np.float32)
    bias = np.asarray(bias, np.float32)

    nc = _get_compiled()

    xs = np.ascontiguousarray(x.T)                     # (d_in, B)
    tbl = _build_tables(y)                             # (8, 128, 8192)
    e_np = _build_selector()
    tc_np = np.full((33, 1), -D_MIN / H, np.float32)
    tc_np[32, 0] = 0.0

    in_maps = []
    for c in range(N_CORES):
        xt = np.empty((33, B), np.float32)
        xt[:32] = xs[c * I_PER:(c + 1) * I_PER]
        xt[32] = H                                     # relu(H/H + 0) == 1.0
        in_maps.append({
            "xt": xt,
            "tbl": tbl[c],
            "sel": e_np,
            "tcol": tc_np,
        })
    res = run_bass_kernel_spmd(nc, in_maps, core_ids=list(range(N_CORES)),
                               trace=trace)
    partialT = np.stack([res.results[c]["out"] for c in range(N_CORES)])
    out = partialT.astype(np.float64).sum(axis=0).T + bias.astype(np.float64)
    return out.astype(np.float32), res


def kernel(x, y, bias):
    out, _ = _run(x, y, bias)
    return out
